# revision 36
# baseline (speedup 1.0000x reference)
"""Trainium2 Bass kernel for nn_AspectModel (span-attention aspect tagger).

Strategy: batch-shard the 32 sentences 4-per-core across 8 NeuronCores; route
each fragment (host-side) to the core owning its sentence, padded to 48 slots
per sentence (192 slots/core; max real count is data-bounded at ~43).

Host stages every operand in its final on-chip dtype and layout:
  - x      [s, d] bf16 (span-feature stationary + mix moving)
  - memT   [d, s] fp8  (pre-transposed copy for the score matmuls)
  - att_w  fp8 (stationary; mixed fp8 x bf16 matmuls), masks fp8/bf16
so the device does no casts and no DMA-xbar transposes.  Spans always live in
positions < 128 (frag_s < 120, width <= 7), so the span masks-matmul contracts
only the first sentence half.  The c row and the span part of the tag logits
are fused into one matmul pass against a host-packed [att_b | tag_w] block.
The attention mix is produced in [slot, d] layout (stationary = u*pw) and
shipped raw; the host applies the tiny tag projection, 1/denominator, tag_b,
and the 5-wide log_softmax.  A PE warm-up burst while the inputs stream keeps
the tensor engine's p-state at full clock for the real work.
"""

import sys
import types

import ml_dtypes
import numpy as np

# Optional shim so run_bass_kernel_spmd(trace=True) works in containers where
# antenv.axon_hooks is missing (profiling only; correctness path unaffected).
try:
    import antenv.axon_hooks  # noqa: F401
except ImportError:
    try:
        from trn_agent_boot.trn_boot import _ntff_profile_via_ctypes

        _hook = _ntff_profile_via_ctypes("/opt/axon/libaxon_pjrt.so")
        _mod = types.ModuleType("antenv.axon_hooks")
        _mod.get_axon_ntff_profile_hook = lambda: _hook
        _mod.set_axon_ntff_profile_hook = lambda h: None
        sys.modules["antenv.axon_hooks"] = _mod
    except Exception:
        pass

import concourse.bass as bass  # noqa: E402
import concourse.tile as tile  # noqa: E402
from concourse import bacc, mybir  # noqa: E402
from concourse import bass_utils  # noqa: E402
from concourse.bass_utils import run_bass_kernel_spmd  # noqa: E402

# No artifact bucket in the sandbox; make tracing's upload step a no-op.
bass_utils.upload_artifacts = lambda tmpdir: f"local:{tmpdir}"

F32 = mybir.dt.float32
BF16 = mybir.dt.bfloat16
F8 = mybir.dt.float8e4
ALU = mybir.AluOpType
ACT = mybir.ActivationFunctionType

B, S, D, F, T = 32, 256, 512, 1024, 5
NCORES = 8
SEN = 4          # sentences per core
G = 48           # fragment slots per sentence
C = SEN * G      # 192 fragment slots per core
KH = 128         # half sentence length (two k-halves per sentence)

NPBF = ml_dtypes.bfloat16
NPF8 = ml_dtypes.float8_e4m3fn

TRACE = False
LAST_RESULT = None  # BassKernelResults of the most recent run (for test.py)

_compiled = {}
DEBUG = False


def _build():
    """Build + compile the per-core SPMD graph (identical on all 8 cores)."""
    nc = bacc.Bacc("TRN2", target_bir_lowering=False, debug=False,
                   num_devices=NCORES)

    x_d = nc.dram_tensor("x", [128, 2, SEN, D], BF16, kind="ExternalInput")
    memT_d = nc.dram_tensor("memT", [128, SEN, 2, 4, KH], F8,
                            kind="ExternalInput")
    aw_d = nc.dram_tensor("aw", [128, 6, 2, D], F8, kind="ExternalInput")
    # awc: per spanT chunk kk, [att_b | tag_w span rows] (6 cols)
    awc_d = nc.dram_tensor("awc", [128, 12, 6], BF16, kind="ExternalInput")
    mk_d = nc.dram_tensor("mk", [128, 3, C], F8, kind="ExternalInput")
    kppw_d = nc.dram_tensor("kppw", [128, 4, C], BF16, kind="ExternalInput")
    out1_d = nc.dram_tensor("out1", [6, C], F32, kind="ExternalOutput")
    out2_d = nc.dram_tensor("out2", [1, C], F32, kind="ExternalOutput")
    # raw attention mix, [slot-packed partitions, l-pair, d] bf16
    out3_d = nc.dram_tensor("out3", [128, 2, D], F8, kind="ExternalOutput")

    with tile.TileContext(nc) as tc:
        with (
            tc.tile_pool(name="persist", bufs=1) as pp,
            tc.tile_pool(name="work", bufs=2) as wp,
            tc.tile_pool(name="psum", bufs=2, space="PSUM") as psp,
        ):
            # ---- persistent SBUF tensors ----
            x_sb = pp.tile([128, 2, SEN, D], BF16, tag="x_sb")
            memT = pp.tile([128, SEN, 2, 4, KH], F8, tag="memT")
            aw_sb = pp.tile([128, 6, 2, D], F8, tag="aw_sb")
            awc_sb = pp.tile([128, 12, 6], BF16, tag="awc_sb")
            mk_sb = pp.tile([128, 3, C], F8, tag="mk_sb")
            kppw = pp.tile([128, 4, C], BF16, tag="kppw")
            warm = pp.tile([128, 256], BF16, tag="warm")
            spanB = pp.tile([128, 12, C], BF16, tag="spanB")
            v_sb = pp.tile([128, 4, C], F8, tag="v_sb")
            c_rb = pp.tile([1, C], BF16, tag="c_rb")
            ones128 = pp.tile([1, 128], BF16, tag="ones128")
            uT = pp.tile([128, 2, C], BF16, tag="uT")
            wTu = pp.tile([128, 2, C], BF16, tag="wTu")
            mix_sb = pp.tile([128, 2, D], F8, tag="mix_sb")
            ones1 = pp.tile([128, 1], BF16, tag="ones1")
            neg4 = pp.tile([128, 1], F32, tag="neg4")
            out1_sb = pp.tile([6, C], F32, tag="out1_sb")
            out2_sb = pp.tile([1, C], F32, tag="out2_sb")

            # ---- input DMAs: one sync queue, ordered by first use ----
            prev = None
            bulk = [(mk_sb[:], mk_d.ap()),
                    (x_sb[:, 0, 0:1], x_d.ap()[:, 0, 0:1]),
                    (x_sb[:, 0, 1:2], x_d.ap()[:, 0, 1:2]),
                    (x_sb[:, 0, 2:4], x_d.ap()[:, 0, 2:4]),
                    (awc_sb[:], awc_d.ap()),
                    (aw_sb[:], aw_d.ap()),
                    (memT[:], memT_d.ap()),
                    (kppw[:], kppw_d.ap()),
                    (x_sb[:, 1], x_d.ap()[:, 1])]
            for dst, src in bulk:
                d = nc.sync.dma_start(dst, src)
                if prev is not None:
                    tile.add_dep_helper(d.ins, prev.ins, sync=False,
                                        reason="bulk ring order")
                prev = d

            # ---- constants ----
            nc.gpsimd.memset(neg4[:], -1.0e4)
            nc.gpsimd.memset(ones1[:], 1.0)
            nc.gpsimd.memset(ones128[:], 1.0)
            nc.gpsimd.memset(warm[:], 0.0)

            # PSUM banks: pv holds vmm (4 banks, one per dj: interleaved
            # accumulation groups must not share a bank) and later the two
            # mix tiles; psm holds spanmm + the fused logits row; pcd holds
            # the score grid + the denominator.
            pv = [psp.tile([128, 512], F32, tag="pv", name=f"pv{dj}", bufs=4)
                  for dj in range(4)]

            # ---- PE warm-up: keep the tensor engine continuously busy while
            # the input DMAs stream so its p-state reaches full clock before
            # the first real matmul (results discarded; pv[0] is reset by
            # vmm's start=True later).
            for w in range(18):
                nc.tensor.matmul(pv[0][:, 0:256], warm[:, 0:128], warm[:],
                                 start=True, stop=True)

            # ---- span masks-matmul (k=0 half only) ----
            # ps[d, (dj, comp, slot)] = sum_s x[s, d] * mask[s, comp, slot]
            sc = nc.named_scope("spanmm"); sc.__enter__()
            evac = [nc.vector.tensor_copy,
                    lambda dst, src: nc.scalar.copy(dst, src)]
            for j0 in range(2):
                for l in range(SEN):
                    ps = psp.tile([128, 2, 3, G], F32, tag="psm", bufs=3)
                    for dj in range(2):
                        j = j0 * 2 + dj
                        nc.tensor.matmul(
                            ps[:, dj], x_sb[:, 0, l, j * 128:(j + 1) * 128],
                            mk_sb[:, 0:3, l * G:(l + 1) * G],
                            start=True, stop=True)
                    dstB = spanB[:, j0 * 6:(j0 + 1) * 6, l * G:(l + 1) * G]
                    evac[(l * 2 + j0) % 2](dstB, ps[:])
            sc.__exit__(None, None, None)

            # ---- v = span @ att_w (fp8 stationary, bf16 moving) ----
            sc = nc.named_scope("vmm"); sc.__enter__()
            for half in range(2):
                for dj in range(4):
                    for kk in range(6 * half, 6 * half + 6):
                        nc.tensor.matmul(
                            pv[dj][:, 0:C],
                            aw_sb[:, kk // 2, kk % 2, dj * 128:(dj + 1) * 128],
                            spanB[:, kk, :],
                            start=(kk == 0), stop=(kk == 11))
            for dj in range(4):
                evac[dj % 2](v_sb[:, dj, :], pv[dj][:, 0:C])
            sc.__exit__(None, None, None)

            # ---- fused c + span-tag-logits: out rows = [c | pls0..4] ----
            sc = nc.named_scope("cmm"); sc.__enter__()
            po = psp.tile([6, C], F32, tag="psm", name="po", bufs=3)
            for kk in range(12):
                nc.tensor.matmul(po[:], awc_sb[:, kk, :], spanB[:, kk, :],
                                 start=(kk == 0), stop=(kk == 11))
            nc.scalar.copy(out1_sb[:], po[:])
            nc.vector.tensor_copy(c_rb[:], po[0:1, :])
            nc.sync.dma_start(out1_d.ap(), out1_sb[:])
            sc.__exit__(None, None, None)

            # ---- scores + masked softmax chain, interleaved by k ----
            sc = nc.named_scope("gts"); sc.__enter__()
            gt = psp.tile([128, 2, C], F32, tag="pcd", name="gt", bufs=1)
            cbc = psp.tile([128, C], F32, tag="psm", name="cbc", bufs=3)
            for k in range(2):
                for l in range(SEN):
                    for dj in range(4):
                        nc.tensor.matmul(
                            gt[:, k, l * G:(l + 1) * G],
                            memT[:, l, k, dj, :],
                            v_sb[:, dj, l * G:(l + 1) * G],
                            start=(dj == 0), stop=(dj == 3))
                if k == 0:
                    nc.tensor.matmul(cbc[:], ones128[:], c_rb[:],
                                     start=True, stop=True)
                sg = wp.tile([128, C], F32, tag="sg", name=f"sg{k}")
                th = wp.tile([128, C], F32, tag="th", name=f"th{k}")
                e0 = wp.tile([128, C], F32, tag="e0", name=f"e0{k}")
                nc.vector.tensor_tensor(sg[:], gt[:, k], kppw[:, 2 + k], op=ALU.mult)
                nc.vector.tensor_tensor(sg[:], sg[:], cbc[:], op=ALU.add)
                nc.scalar.activation(th[:], sg[:], ACT.Tanh)
                nc.scalar.activation(e0[:], th[:], ACT.Exp)
                nc.vector.tensor_tensor(wTu[:, k], e0[:], kppw[:, 2 + k],
                                        op=ALU.mult)
                nc.vector.tensor_tensor(uT[:, k], e0[:], kppw[:, k],
                                        op=ALU.mult)
            sc.__exit__(None, None, None)

            # ---- softmax denominator (ships to host; host divides) ----
            pd = psp.tile([1, C], F32, tag="pcd", name="pd", bufs=1)
            for k in range(2):
                nc.tensor.matmul(pd[:], ones1[:], uT[:, k], start=(k == 0),
                                 stop=(k == 1))
            nc.vector.tensor_copy(out2_sb[:], pd[:])
            nc.sync.dma_start(out2_d.ap(), out2_sb[:])

            # ---- mix[slot, d] = sum_{s,k} wTu[s, k, slot] x[s, k, d] ----
            # stationary = wTu block (48 slot cols), moving = full x row;
            # sentence l lands at partitions (l%2)*64 .. +48 of tile l//2.
            sc = nc.named_scope("mix"); sc.__enter__()
            pmix = [psp.tile([128, 512], F32, tag="pv", name=f"pmix{t}", bufs=4)
                    for t in range(2)]
            for t in range(2):
                for l in (2 * t, 2 * t + 1):
                    for k in range(2):
                        nc.tensor.matmul(
                            pmix[t][(l % 2) * 64:(l % 2) * 64 + G, :],
                            wTu[:, k, l * G:(l + 1) * G],
                            x_sb[:, k, l, :],
                            start=(k == 0), stop=(k == 1))
                for h in range(2):
                    evac[h](mix_sb[:, t, h * 256:(h + 1) * 256],
                            pmix[t][:, h * 256:(h + 1) * 256])
                nc.sync.dma_start(out3_d.ap()[:, t, :], mix_sb[:, t, :])
            sc.__exit__(None, None, None)

    nc.compile()
    return nc


def _row_of(kk, p):
    """att row for spanT chunk kk at partition p: comp*D + j*128 + p."""
    return (kk % 3) * D + (kk // 3) * 128 + p


def _host_prep(en_output, lengths, frag_b, frag_s, frag_e, att_w, att_b,
               tag_w, tag_b):
    """Shard + relayout inputs.  Returns (in_maps, assign, overflow)."""
    seq = float(lengths[0])
    p = np.arange(128)
    kk12 = np.arange(12)
    rows = _row_of(kk12[None, :], p[:, None])            # [128, 12]
    aw_np = np.ascontiguousarray(
        att_w[rows].reshape(128, 6, 2, D)).astype(NPF8)
    awc = np.empty((128, 12, 6), np.float32)
    awc[:, :, 0] = att_b[rows]
    awc[:, :, 1:] = tag_w[:, rows].transpose(1, 2, 0)    # [128, 12, T]
    awc_np = np.ascontiguousarray(awc).astype(NPBF)

    assign = np.full((F, 2), -1, dtype=np.int64)  # (core, slot) per fragment
    counts = np.zeros((NCORES, SEN), dtype=np.int64)
    overflow = []
    fs_slot = np.zeros((NCORES, C), np.float32)
    fm_slot = np.full((NCORES, C), -1.0, np.float32)
    ln_slot = np.full((NCORES, C), float(S), np.float32)

    for i in range(F):
        b = int(frag_b[i])
        s, e = int(frag_s[i]), int(frag_e[i])
        core, l = b // SEN, b % SEN
        k = counts[core, l]
        if k >= G or s >= KH or e > KH:
            overflow.append(i)
            continue
        counts[core, l] += 1
        slot = l * G + k
        assign[i] = (core, slot)
        fs_slot[core, slot] = s
        fm_slot[core, slot] = e - 1
        ln_slot[core, slot] = lengths[b]

    in_maps = []
    s0 = p.astype(np.float32)[:, None]                   # [128, 1]
    for core in range(NCORES):
        fs = fs_slot[core][None, :]
        fm = fm_slot[core][None, :]
        ln = ln_slot[core][None, :]
        mk = np.zeros((128, 3, C), np.float32)
        mk[:, 0, :] = s0 == fs
        mk[:, 1, :] = (s0 >= fs) & (s0 <= fm)
        mk[:, 2, :] = s0 == fm
        kppw = np.empty((128, 4, C), np.float32)
        for k in range(2):
            s = k * KH + s0
            keep = (~((s >= fs) & (s <= fm))) & (s < ln)
            kppw[:, k, :] = keep
            dis = np.where(s < fs, fs - s, np.where(s > fm, s - fm, seq))
            kppw[:, 2 + k, :] = (1.0 - dis / seq) * keep
        xs = en_output[core * SEN:(core + 1) * SEN]      # [4, 256, 512]
        x_np = np.ascontiguousarray(
            xs.reshape(SEN, 2, 128, D).transpose(2, 1, 0, 3)).astype(NPBF)
        mem_np = np.ascontiguousarray(
            xs.reshape(SEN, 2, 128, 4, 128).transpose(4, 0, 1, 3, 2)).astype(NPF8)
        in_maps.append({
            "x": x_np, "memT": mem_np, "aw": aw_np, "awc": awc_np,
            "mk": np.ascontiguousarray(mk).astype(NPF8),
            "kppw": np.ascontiguousarray(kppw).astype(NPBF),
        })
    return in_maps, assign, overflow


def _host_fragment(en_output, lengths, s, e, b, att_w, att_b, tag_w, tag_b,
                   seq_len):
    """Numpy fallback for (vanishingly rare) slot-overflow fragments."""
    mem = en_output[b].astype(np.float64)
    ws = mem[s:e].sum(0)
    span = np.concatenate([mem[s], ws, mem[e - 1]])
    pos = np.arange(S)
    in_span = (pos >= s) & (pos < e)
    att_mask = in_span | (pos >= lengths[b])
    dis = np.where(pos < s, s - pos,
                   np.where(pos >= e, pos - e + 1, seq_len)).astype(np.float64)
    pwv = 1.0 - dis / seq_len
    fin = pwv[:, None] * mem
    v = span @ att_w.astype(np.float64)
    c = span @ att_b.astype(np.float64)
    sc = np.tanh(fin @ v + c)
    sc = np.where(att_mask, -1e4, sc)
    sc = sc - sc.max()
    a = np.exp(sc)
    a = a / a.sum()
    mix = a @ fin
    ms = np.concatenate([span, mix])
    lg = ms @ tag_w.astype(np.float64).T + tag_b.astype(np.float64)
    lg = lg - lg.max()
    return (lg - np.log(np.exp(lg).sum())).astype(np.float32)


def kernel(en_output, lengths, frag_b, frag_s, frag_e, att_w, att_b, tag_w,
           tag_b):
    global LAST_RESULT
    en_output = np.asarray(en_output, dtype=np.float32)
    lengths = np.asarray(lengths).astype(np.int64)
    frag_b = np.asarray(frag_b).astype(np.int64)
    frag_s = np.asarray(frag_s).astype(np.int64)
    frag_e = np.asarray(frag_e).astype(np.int64)
    att_w = np.asarray(att_w, dtype=np.float32)
    att_b = np.asarray(att_b, dtype=np.float32)
    tag_w = np.asarray(tag_w, dtype=np.float32)
    tag_b = np.asarray(tag_b, dtype=np.float32)

    if "nc" not in _compiled:
        _compiled["nc"] = _build()
    nc = _compiled["nc"]

    in_maps, assign, overflow = _host_prep(
        en_output, lengths, frag_b, frag_s, frag_e, att_w, att_b, tag_w, tag_b)

    res = run_bass_kernel_spmd(nc, in_maps, core_ids=list(range(NCORES)),
                               trace=TRACE)
    LAST_RESULT = res

    tw_mix = tag_w[:, 3 * D:]                            # [T, D]
    out = np.empty((F, T), dtype=np.float32)
    per_core = []
    for i in range(NCORES):
        o1 = np.asarray(res.results[i]["out1"], np.float32)     # [6, C]
        dn = np.asarray(res.results[i]["out2"], np.float32)[0]  # [C]
        mx = np.asarray(res.results[i]["out3"], np.float32)     # [128, 2, D]
        pls = o1[1:6, :].T                                      # [C, T]
        mix = np.empty((C, D), np.float32)
        for l in range(SEN):
            mix[l * G:(l + 1) * G] = mx[(l % 2) * 64:(l % 2) * 64 + G, l // 2]
        lg = pls + (mix @ tw_mix.T) / dn[:, None] + tag_b[None, :]
        m = lg.max(axis=1, keepdims=True)
        ls = lg - m
        per_core.append(ls - np.log(np.exp(ls).sum(axis=1, keepdims=True)))
    cores = assign[:, 0]
    slots = assign[:, 1]
    for core in range(NCORES):
        sel = cores == core
        out[sel] = per_core[core][slots[sel]]
    seq_len = float(lengths[0])
    for i in overflow:
        out[i] = _host_fragment(en_output, lengths, int(frag_s[i]),
                                int(frag_e[i]), int(frag_b[i]), att_w, att_b,
                                tag_w, tag_b, seq_len)
    return out


# revision 37
# speedup vs baseline: 1.0935x; 1.0935x over previous
"""Trainium2 Bass kernel for nn_AspectModel (span-attention aspect tagger).

Strategy: batch-shard the 32 sentences 4-per-core across 8 NeuronCores; route
each fragment (host-side) to the core owning its sentence, padded to 48 slots
per sentence (192 slots/core; max real count is data-bounded at ~43).

Host stages every operand in its final on-chip dtype and layout:
  - x      [s, d] bf16 (span-feature stationary + mix moving)
  - memT   [d, s] fp8  (pre-transposed copy for the score matmuls)
  - att_w  fp8 (stationary; mixed fp8 x bf16 matmuls), masks fp8/bf16
so the device does no casts and no DMA-xbar transposes.  Spans always live in
positions < 128 (frag_s < 120, width <= 7), so the span masks-matmul contracts
only the first sentence half.  The c row and the span part of the tag logits
are fused into one matmul pass against a host-packed [att_b | tag_w] block.
The attention mix is produced in [slot, d] layout (stationary = u*pw) and
shipped raw; the host applies the tiny tag projection, 1/denominator, tag_b,
and the 5-wide log_softmax.  A PE warm-up burst while the inputs stream keeps
the tensor engine's p-state at full clock for the real work.
"""

import sys
import types

import ml_dtypes
import numpy as np

# Optional shim so run_bass_kernel_spmd(trace=True) works in containers where
# antenv.axon_hooks is missing (profiling only; correctness path unaffected).
try:
    import antenv.axon_hooks  # noqa: F401
except ImportError:
    try:
        from trn_agent_boot.trn_boot import _ntff_profile_via_ctypes

        _hook = _ntff_profile_via_ctypes("/opt/axon/libaxon_pjrt.so")
        _mod = types.ModuleType("antenv.axon_hooks")
        _mod.get_axon_ntff_profile_hook = lambda: _hook
        _mod.set_axon_ntff_profile_hook = lambda h: None
        sys.modules["antenv.axon_hooks"] = _mod
    except Exception:
        pass

import concourse.bass as bass  # noqa: E402
import concourse.tile as tile  # noqa: E402
from concourse import bacc, mybir  # noqa: E402
from concourse import bass_utils  # noqa: E402
from concourse.bass_utils import run_bass_kernel_spmd  # noqa: E402

# No artifact bucket in the sandbox; make tracing's upload step a no-op.
bass_utils.upload_artifacts = lambda tmpdir: f"local:{tmpdir}"

F32 = mybir.dt.float32
BF16 = mybir.dt.bfloat16
F8 = mybir.dt.float8e4
ALU = mybir.AluOpType
ACT = mybir.ActivationFunctionType

B, S, D, F, T = 32, 256, 512, 1024, 5
NCORES = 8
SEN = 4          # sentences per core
G = 48           # fragment slots per sentence
C = SEN * G      # 192 fragment slots per core
KH = 128         # half sentence length (two k-halves per sentence)

NPBF = ml_dtypes.bfloat16
NPF8 = ml_dtypes.float8_e4m3fn

TRACE = False
LAST_RESULT = None  # BassKernelResults of the most recent run (for test.py)

_compiled = {}
DEBUG = False


def _build():
    """Build + compile the per-core SPMD graph (identical on all 8 cores)."""
    nc = bacc.Bacc("TRN2", target_bir_lowering=False, debug=False,
                   num_devices=NCORES)

    x_d = nc.dram_tensor("x", [128, 2, SEN, D], BF16, kind="ExternalInput")
    memT_d = nc.dram_tensor("memT", [128, SEN, 2, 4, KH], F8,
                            kind="ExternalInput")
    aw_d = nc.dram_tensor("aw", [128, 6, 2, D], F8, kind="ExternalInput")
    # awc: per spanT chunk kk, [att_b | tag_w span rows] (6 cols)
    awc_d = nc.dram_tensor("awc", [128, 12, 6], BF16, kind="ExternalInput")
    mk_d = nc.dram_tensor("mk", [128, 3, C], F8, kind="ExternalInput")
    kp_d = nc.dram_tensor("kp", [128, 2, C], BF16, kind="ExternalInput")
    pw_d = nc.dram_tensor("pw", [128, 2, C], BF16, kind="ExternalInput")
    out1_d = nc.dram_tensor("out1", [6, C], F32, kind="ExternalOutput")
    out2_d = nc.dram_tensor("out2", [1, C], F32, kind="ExternalOutput")
    # raw attention mix, [slot-packed partitions, l-pair, d] bf16
    out3_d = nc.dram_tensor("out3", [128, 2, D], F8, kind="ExternalOutput")

    with tile.TileContext(nc) as tc:
        with (
            tc.tile_pool(name="persist", bufs=1) as pp,
            tc.tile_pool(name="work", bufs=2) as wp,
            tc.tile_pool(name="psum", bufs=2, space="PSUM") as psp,
        ):
            # ---- persistent SBUF tensors ----
            x_sb = pp.tile([128, 2, SEN, D], BF16, tag="x_sb")
            memT = pp.tile([128, SEN, 2, 4, KH], F8, tag="memT")
            aw_sb = pp.tile([128, 6, 2, D], F8, tag="aw_sb")
            awc_sb = pp.tile([128, 12, 6], BF16, tag="awc_sb")
            mk_sb = pp.tile([128, 3, C], F8, tag="mk_sb")
            kp_sb = pp.tile([128, 2, C], BF16, tag="kp_sb")
            pw_sb = pp.tile([128, 2, C], BF16, tag="pw_sb")
            warm = pp.tile([128, 256], BF16, tag="warm")
            spanB = pp.tile([128, 12, C], BF16, tag="spanB")
            v_sb = pp.tile([128, 4, C], F8, tag="v_sb")
            c_rb = pp.tile([1, C], BF16, tag="c_rb")
            ones128 = pp.tile([1, 128], BF16, tag="ones128")
            uT = pp.tile([128, 2, C], BF16, tag="uT")
            wTu = pp.tile([128, 2, C], BF16, tag="wTu")
            mix_sb = pp.tile([128, 2, D], F8, tag="mix_sb")
            ones1 = pp.tile([128, 1], BF16, tag="ones1")
            neg4 = pp.tile([128, 1], F32, tag="neg4")
            out1_sb = pp.tile([6, C], F32, tag="out1_sb")
            out2_sb = pp.tile([1, C], F32, tag="out2_sb")

            # ---- input DMAs: one sync queue, ordered by first use ----
            prev = None
            bulk = [(mk_sb[:], mk_d.ap())]
            bulk += [(x_sb[:, 0, l:l + 1], x_d.ap()[:, 0, l:l + 1])
                     for l in range(SEN)]
            bulk += [(awc_sb[:], awc_d.ap())]
            bulk += [(aw_sb[:, 2 * g:2 * g + 2], aw_d.ap()[:, 2 * g:2 * g + 2])
                     for g in range(3)]
            bulk += [(memT[:], memT_d.ap()),
                     (kp_sb[:], kp_d.ap()),
                     (pw_sb[:], pw_d.ap()),
                     (x_sb[:, 1], x_d.ap()[:, 1])]
            for dst, src in bulk:
                d = nc.sync.dma_start(dst, src)
                if prev is not None:
                    tile.add_dep_helper(d.ins, prev.ins, sync=False,
                                        reason="bulk ring order")
                prev = d

            # ---- constants ----
            nc.gpsimd.memset(neg4[:], -1.0e4)
            nc.gpsimd.memset(ones1[:], 1.0)
            nc.gpsimd.memset(ones128[:], 1.0)
            nc.gpsimd.memset(warm[:], 0.0)

            # PSUM banks: pv holds vmm (4 banks, one per dj: interleaved
            # accumulation groups must not share a bank) and later the two
            # mix tiles; psm holds spanmm + the fused logits row; pcd holds
            # the score grid + the denominator.
            pv = [psp.tile([128, 512], F32, tag="pv", name=f"pv{dj}", bufs=4)
                  for dj in range(4)]

            # ---- PE warm-up: keep the tensor engine continuously busy while
            # the input DMAs stream so its p-state reaches full clock before
            # the first real matmul (results discarded; pv[0] is reset by
            # vmm's start=True later).
            for w in range(18):
                nc.tensor.matmul(pv[0][:, 0:256], warm[:, 0:128], warm[:],
                                 start=True, stop=True)

            # ---- span masks-matmul (k=0 half only) ----
            # ps[d, (dj, comp, slot)] = sum_s x[s, d] * mask[s, comp, slot]
            sc = nc.named_scope("spanmm"); sc.__enter__()
            evac = [nc.vector.tensor_copy,
                    lambda dst, src: nc.scalar.copy(dst, src)]
            for j0 in range(2):
                for l in range(SEN):
                    ps = psp.tile([128, 2, 3, G], F32, tag="psm", bufs=3)
                    for dj in range(2):
                        j = j0 * 2 + dj
                        nc.tensor.matmul(
                            ps[:, dj], x_sb[:, 0, l, j * 128:(j + 1) * 128],
                            mk_sb[:, 0:3, l * G:(l + 1) * G],
                            start=True, stop=True)
                    dstB = spanB[:, j0 * 6:(j0 + 1) * 6, l * G:(l + 1) * G]
                    evac[(l * 2 + j0) % 2](dstB, ps[:])
            sc.__exit__(None, None, None)

            # ---- v = span @ att_w (fp8 stationary, bf16 moving) ----
            sc = nc.named_scope("vmm"); sc.__enter__()
            for half in range(2):
                for dj in range(4):
                    for kk in range(6 * half, 6 * half + 6):
                        nc.tensor.matmul(
                            pv[dj][:, 0:C],
                            aw_sb[:, kk // 2, kk % 2, dj * 128:(dj + 1) * 128],
                            spanB[:, kk, :],
                            start=(kk == 0), stop=(kk == 11))
            for dj in range(4):
                evac[dj % 2](v_sb[:, dj, :], pv[dj][:, 0:C])
            sc.__exit__(None, None, None)

            # ---- fused c + span-tag-logits: out rows = [c | pls0..4] ----
            sc = nc.named_scope("cmm"); sc.__enter__()
            po = psp.tile([6, C], F32, tag="psm", name="po", bufs=3)
            for kk in range(12):
                nc.tensor.matmul(po[:], awc_sb[:, kk, :], spanB[:, kk, :],
                                 start=(kk == 0), stop=(kk == 11))
            nc.scalar.copy(out1_sb[:], po[:])
            nc.vector.tensor_copy(c_rb[:], po[0:1, :])
            nc.sync.dma_start(out1_d.ap(), out1_sb[:])
            sc.__exit__(None, None, None)

            # ---- scores + masked softmax chain, interleaved by k ----
            sc = nc.named_scope("gts"); sc.__enter__()
            gt = psp.tile([128, 2, C], F32, tag="pcd", name="gt", bufs=1)
            cbc = psp.tile([128, C], F32, tag="psm", name="cbc", bufs=3)
            for k in range(2):
                for l in range(SEN):
                    for dj in range(4):
                        nc.tensor.matmul(
                            gt[:, k, l * G:(l + 1) * G],
                            memT[:, l, k, dj, :],
                            v_sb[:, dj, l * G:(l + 1) * G],
                            start=(dj == 0), stop=(dj == 3))
                if k == 0:
                    nc.tensor.matmul(cbc[:], ones128[:], c_rb[:],
                                     start=True, stop=True)
                sg = wp.tile([128, C], F32, tag="sg", name=f"sg{k}")
                th = wp.tile([128, C], F32, tag="th", name=f"th{k}")
                e0 = wp.tile([128, C], F32, tag="e0", name=f"e0{k}")
                nc.vector.tensor_tensor(sg[:], gt[:, k], pw_sb[:, k], op=ALU.mult)
                nc.vector.tensor_tensor(sg[:], sg[:], cbc[:], op=ALU.add)
                nc.scalar.activation(th[:], sg[:], ACT.Tanh)
                nc.scalar.activation(e0[:], th[:], ACT.Exp)
                nc.vector.tensor_tensor(wTu[:, k], e0[:], pw_sb[:, k],
                                        op=ALU.mult)
                nc.vector.tensor_tensor(uT[:, k], e0[:], kp_sb[:, k],
                                        op=ALU.mult)
            sc.__exit__(None, None, None)

            # ---- softmax denominator (ships to host; host divides) ----
            pd = psp.tile([1, C], F32, tag="pcd", name="pd", bufs=1)
            for k in range(2):
                nc.tensor.matmul(pd[:], ones1[:], uT[:, k], start=(k == 0),
                                 stop=(k == 1))
            nc.vector.tensor_copy(out2_sb[:], pd[:])
            nc.sync.dma_start(out2_d.ap(), out2_sb[:])

            # ---- mix[slot, d] = sum_{s,k} wTu[s, k, slot] x[s, k, d] ----
            # stationary = wTu block (48 slot cols), moving = full x row;
            # sentence l lands at partitions (l%2)*64 .. +48 of tile l//2.
            sc = nc.named_scope("mix"); sc.__enter__()
            pmix = [psp.tile([128, 512], F32, tag="pv", name=f"pmix{t}", bufs=4)
                    for t in range(2)]
            for t in range(2):
                for l in (2 * t, 2 * t + 1):
                    for k in range(2):
                        nc.tensor.matmul(
                            pmix[t][(l % 2) * 64:(l % 2) * 64 + G, :],
                            wTu[:, k, l * G:(l + 1) * G],
                            x_sb[:, k, l, :],
                            start=(k == 0), stop=(k == 1))
                for h in range(2):
                    evac[h](mix_sb[:, t, h * 256:(h + 1) * 256],
                            pmix[t][:, h * 256:(h + 1) * 256])
                nc.sync.dma_start(out3_d.ap()[:, t, :], mix_sb[:, t, :])
            sc.__exit__(None, None, None)

    nc.compile()
    return nc


def _row_of(kk, p):
    """att row for spanT chunk kk at partition p: comp*D + j*128 + p."""
    return (kk % 3) * D + (kk // 3) * 128 + p


def _host_prep(en_output, lengths, frag_b, frag_s, frag_e, att_w, att_b,
               tag_w, tag_b):
    """Shard + relayout inputs.  Returns (in_maps, assign, overflow)."""
    seq = float(lengths[0])
    p = np.arange(128)
    kk12 = np.arange(12)
    rows = _row_of(kk12[None, :], p[:, None])            # [128, 12]
    aw_np = np.ascontiguousarray(
        att_w[rows].reshape(128, 6, 2, D)).astype(NPF8)
    awc = np.empty((128, 12, 6), np.float32)
    awc[:, :, 0] = att_b[rows]
    awc[:, :, 1:] = tag_w[:, rows].transpose(1, 2, 0)    # [128, 12, T]
    awc_np = np.ascontiguousarray(awc).astype(NPBF)

    assign = np.full((F, 2), -1, dtype=np.int64)  # (core, slot) per fragment
    counts = np.zeros((NCORES, SEN), dtype=np.int64)
    overflow = []
    fs_slot = np.zeros((NCORES, C), np.float32)
    fm_slot = np.full((NCORES, C), -1.0, np.float32)
    ln_slot = np.full((NCORES, C), float(S), np.float32)

    for i in range(F):
        b = int(frag_b[i])
        s, e = int(frag_s[i]), int(frag_e[i])
        core, l = b // SEN, b % SEN
        k = counts[core, l]
        if k >= G or s >= KH or e > KH:
            overflow.append(i)
            continue
        counts[core, l] += 1
        slot = l * G + k
        assign[i] = (core, slot)
        fs_slot[core, slot] = s
        fm_slot[core, slot] = e - 1
        ln_slot[core, slot] = lengths[b]

    in_maps = []
    s0 = p.astype(np.float32)[:, None]                   # [128, 1]
    for core in range(NCORES):
        fs = fs_slot[core][None, :]
        fm = fm_slot[core][None, :]
        ln = ln_slot[core][None, :]
        mk = np.zeros((128, 3, C), np.float32)
        mk[:, 0, :] = s0 == fs
        mk[:, 1, :] = (s0 >= fs) & (s0 <= fm)
        mk[:, 2, :] = s0 == fm
        kp = np.empty((128, 2, C), np.float32)
        pw = np.empty((128, 2, C), np.float32)
        for k in range(2):
            s = k * KH + s0
            keep = (~((s >= fs) & (s <= fm))) & (s < ln)
            kp[:, k, :] = keep
            dis = np.where(s < fs, fs - s, np.where(s > fm, s - fm, seq))
            pw[:, k, :] = (1.0 - dis / seq) * keep
        xs = en_output[core * SEN:(core + 1) * SEN]      # [4, 256, 512]
        x_np = np.ascontiguousarray(
            xs.reshape(SEN, 2, 128, D).transpose(2, 1, 0, 3)).astype(NPBF)
        mem_np = np.ascontiguousarray(
            xs.reshape(SEN, 2, 128, 4, 128).transpose(4, 0, 1, 3, 2)).astype(NPF8)
        in_maps.append({
            "x": x_np, "memT": mem_np, "aw": aw_np, "awc": awc_np,
            "mk": np.ascontiguousarray(mk).astype(NPF8),
            "kp": np.ascontiguousarray(kp).astype(NPBF),
            "pw": np.ascontiguousarray(pw).astype(NPBF),
        })
    return in_maps, assign, overflow


def _host_fragment(en_output, lengths, s, e, b, att_w, att_b, tag_w, tag_b,
                   seq_len):
    """Numpy fallback for (vanishingly rare) slot-overflow fragments."""
    mem = en_output[b].astype(np.float64)
    ws = mem[s:e].sum(0)
    span = np.concatenate([mem[s], ws, mem[e - 1]])
    pos = np.arange(S)
    in_span = (pos >= s) & (pos < e)
    att_mask = in_span | (pos >= lengths[b])
    dis = np.where(pos < s, s - pos,
                   np.where(pos >= e, pos - e + 1, seq_len)).astype(np.float64)
    pwv = 1.0 - dis / seq_len
    fin = pwv[:, None] * mem
    v = span @ att_w.astype(np.float64)
    c = span @ att_b.astype(np.float64)
    sc = np.tanh(fin @ v + c)
    sc = np.where(att_mask, -1e4, sc)
    sc = sc - sc.max()
    a = np.exp(sc)
    a = a / a.sum()
    mix = a @ fin
    ms = np.concatenate([span, mix])
    lg = ms @ tag_w.astype(np.float64).T + tag_b.astype(np.float64)
    lg = lg - lg.max()
    return (lg - np.log(np.exp(lg).sum())).astype(np.float32)


def kernel(en_output, lengths, frag_b, frag_s, frag_e, att_w, att_b, tag_w,
           tag_b):
    global LAST_RESULT
    en_output = np.asarray(en_output, dtype=np.float32)
    lengths = np.asarray(lengths).astype(np.int64)
    frag_b = np.asarray(frag_b).astype(np.int64)
    frag_s = np.asarray(frag_s).astype(np.int64)
    frag_e = np.asarray(frag_e).astype(np.int64)
    att_w = np.asarray(att_w, dtype=np.float32)
    att_b = np.asarray(att_b, dtype=np.float32)
    tag_w = np.asarray(tag_w, dtype=np.float32)
    tag_b = np.asarray(tag_b, dtype=np.float32)

    if "nc" not in _compiled:
        _compiled["nc"] = _build()
    nc = _compiled["nc"]

    in_maps, assign, overflow = _host_prep(
        en_output, lengths, frag_b, frag_s, frag_e, att_w, att_b, tag_w, tag_b)

    res = run_bass_kernel_spmd(nc, in_maps, core_ids=list(range(NCORES)),
                               trace=TRACE)
    LAST_RESULT = res

    tw_mix = tag_w[:, 3 * D:]                            # [T, D]
    out = np.empty((F, T), dtype=np.float32)
    per_core = []
    for i in range(NCORES):
        o1 = np.asarray(res.results[i]["out1"], np.float32)     # [6, C]
        dn = np.asarray(res.results[i]["out2"], np.float32)[0]  # [C]
        mx = np.asarray(res.results[i]["out3"], np.float32)     # [128, 2, D]
        pls = o1[1:6, :].T                                      # [C, T]
        mix = np.empty((C, D), np.float32)
        for l in range(SEN):
            mix[l * G:(l + 1) * G] = mx[(l % 2) * 64:(l % 2) * 64 + G, l // 2]
        lg = pls + (mix @ tw_mix.T) / dn[:, None] + tag_b[None, :]
        m = lg.max(axis=1, keepdims=True)
        ls = lg - m
        per_core.append(ls - np.log(np.exp(ls).sum(axis=1, keepdims=True)))
    cores = assign[:, 0]
    slots = assign[:, 1]
    for core in range(NCORES):
        sel = cores == core
        out[sel] = per_core[core][slots[sel]]
    seq_len = float(lengths[0])
    for i in overflow:
        out[i] = _host_fragment(en_output, lengths, int(frag_s[i]),
                                int(frag_e[i]), int(frag_b[i]), att_w, att_b,
                                tag_w, tag_b, seq_len)
    return out


# revision 38
# speedup vs baseline: 1.1413x; 1.0437x over previous
"""Trainium2 Bass kernel for nn_AspectModel (span-attention aspect tagger).

Strategy: batch-shard the 32 sentences 4-per-core across 8 NeuronCores; route
each fragment (host-side) to the core owning its sentence, padded to 48 slots
per sentence (192 slots/core; max real count is data-bounded at ~43).

Host stages every operand in its final on-chip dtype and layout:
  - x      [s, d] bf16 (span-feature stationary + mix moving)
  - memT   [d, s] fp8  (pre-transposed copy for the score matmuls)
  - att_w  fp8 (stationary; mixed fp8 x bf16 matmuls), masks fp8/bf16
so the device does no casts and no DMA-xbar transposes.  Spans always live in
positions < 128 (frag_s < 120, width <= 7), so the span masks-matmul contracts
only the first sentence half.  The c row and the span part of the tag logits
are fused into one matmul pass against a host-packed [att_b | tag_w] block.
The attention mix is produced in [slot, d] layout (stationary = u*pw) and
shipped raw; the host applies the tiny tag projection, 1/denominator, tag_b,
and the 5-wide log_softmax.  A PE warm-up burst while the inputs stream keeps
the tensor engine's p-state at full clock for the real work.
"""

import sys
import types

import ml_dtypes
import numpy as np

# Optional shim so run_bass_kernel_spmd(trace=True) works in containers where
# antenv.axon_hooks is missing (profiling only; correctness path unaffected).
try:
    import antenv.axon_hooks  # noqa: F401
except ImportError:
    try:
        from trn_agent_boot.trn_boot import _ntff_profile_via_ctypes

        _hook = _ntff_profile_via_ctypes("/opt/axon/libaxon_pjrt.so")
        _mod = types.ModuleType("antenv.axon_hooks")
        _mod.get_axon_ntff_profile_hook = lambda: _hook
        _mod.set_axon_ntff_profile_hook = lambda h: None
        sys.modules["antenv.axon_hooks"] = _mod
    except Exception:
        pass

import concourse.bass as bass  # noqa: E402
import concourse.tile as tile  # noqa: E402
from concourse import bacc, mybir  # noqa: E402
from concourse import bass_utils  # noqa: E402
from concourse.bass_utils import run_bass_kernel_spmd  # noqa: E402

# No artifact bucket in the sandbox; make tracing's upload step a no-op.
bass_utils.upload_artifacts = lambda tmpdir: f"local:{tmpdir}"

F32 = mybir.dt.float32
BF16 = mybir.dt.bfloat16
F8 = mybir.dt.float8e4
ALU = mybir.AluOpType
ACT = mybir.ActivationFunctionType

B, S, D, F, T = 32, 256, 512, 1024, 5
NCORES = 8
SEN = 4          # sentences per core
G = 48           # fragment slots per sentence
C = SEN * G      # 192 fragment slots per core
KH = 128         # half sentence length (two k-halves per sentence)

NPBF = ml_dtypes.bfloat16
NPF8 = ml_dtypes.float8_e4m3fn

TRACE = False
LAST_RESULT = None  # BassKernelResults of the most recent run (for test.py)

_compiled = {}
DEBUG = False


def _build():
    """Build + compile the per-core SPMD graph (identical on all 8 cores)."""
    nc = bacc.Bacc("TRN2", target_bir_lowering=False, debug=False,
                   num_devices=NCORES)

    x_d = nc.dram_tensor("x", [128, 2, SEN, D], BF16, kind="ExternalInput")
    memT_d = nc.dram_tensor("memT", [128, SEN, 2, 4, KH], F8,
                            kind="ExternalInput")
    aw_d = nc.dram_tensor("aw", [128, 6, 2, D], F8, kind="ExternalInput")
    # awc: per spanT chunk kk, [att_b | tag_w span rows] (6 cols)
    awc_d = nc.dram_tensor("awc", [128, 12, 6], BF16, kind="ExternalInput")
    mk_d = nc.dram_tensor("mk", [128, 3, C], F8, kind="ExternalInput")
    kp_d = nc.dram_tensor("kp", [128, 2, C], BF16, kind="ExternalInput")
    pw_d = nc.dram_tensor("pw", [128, 2, C], BF16, kind="ExternalInput")
    out1_d = nc.dram_tensor("out1", [6, C], F32, kind="ExternalOutput")
    out2_d = nc.dram_tensor("out2", [1, C], F32, kind="ExternalOutput")
    # raw attention mix, [slot-packed partitions, l-pair, d] bf16
    out3_d = nc.dram_tensor("out3", [128, 2, D], F8, kind="ExternalOutput")

    with tile.TileContext(nc) as tc:
        with (
            tc.tile_pool(name="persist", bufs=1) as pp,
            tc.tile_pool(name="work", bufs=2) as wp,
            tc.tile_pool(name="psum", bufs=2, space="PSUM") as psp,
        ):
            # ---- persistent SBUF tensors ----
            x_sb = pp.tile([128, 2, SEN, D], BF16, tag="x_sb")
            memT = pp.tile([128, SEN, 2, 4, KH], F8, tag="memT")
            aw_sb = pp.tile([128, 6, 2, D], F8, tag="aw_sb")
            awc_sb = pp.tile([128, 12, 6], BF16, tag="awc_sb")
            mk_sb = pp.tile([128, 3, C], F8, tag="mk_sb")
            kp_sb = pp.tile([128, 2, C], BF16, tag="kp_sb")
            pw_sb = pp.tile([128, 2, C], BF16, tag="pw_sb")
            warm = pp.tile([128, 256], BF16, tag="warm")
            spanB = pp.tile([128, 12, C], BF16, tag="spanB")
            v_sb = pp.tile([128, 4, C], F8, tag="v_sb")
            c_rb = pp.tile([1, C], BF16, tag="c_rb")
            ones128 = pp.tile([1, 128], BF16, tag="ones128")
            uT = pp.tile([128, 2, C], BF16, tag="uT")
            wTu = pp.tile([128, 2, C], BF16, tag="wTu")
            mix_sb = pp.tile([128, 2, D], F8, tag="mix_sb")
            ones1 = pp.tile([128, 1], BF16, tag="ones1")
            neg4 = pp.tile([128, 1], F32, tag="neg4")
            out1_sb = pp.tile([6, C], F32, tag="out1_sb")
            out2_sb = pp.tile([1, C], F32, tag="out2_sb")

            # ---- input DMAs: one sync queue, ordered by first use ----
            prev = None
            bulk = [(mk_sb[:], mk_d.ap())]
            bulk += [(x_sb[:, 0, l:l + 1], x_d.ap()[:, 0, l:l + 1])
                     for l in range(SEN)]
            bulk += [(awc_sb[:], awc_d.ap())]
            bulk += [(aw_sb[:, 2 * g:2 * g + 2], aw_d.ap()[:, 2 * g:2 * g + 2])
                     for g in range(3)]
            bulk += [(memT[:], memT_d.ap()),
                     (kp_sb[:], kp_d.ap()),
                     (pw_sb[:], pw_d.ap()),
                     (x_sb[:, 1], x_d.ap()[:, 1])]
            for dst, src in bulk:
                d = nc.sync.dma_start(dst, src)
                if prev is not None:
                    tile.add_dep_helper(d.ins, prev.ins, sync=False,
                                        reason="bulk ring order")
                prev = d

            # ---- constants ----
            nc.gpsimd.memset(neg4[:], -1.0e4)
            nc.gpsimd.memset(ones1[:], 1.0)
            nc.gpsimd.memset(ones128[:], 1.0)
            nc.gpsimd.memset(warm[:], 0.0)

            # PSUM banks: pv holds vmm (4 banks, one per dj: interleaved
            # accumulation groups must not share a bank) and later the two
            # mix tiles; psm holds spanmm + the fused logits row; pcd holds
            # the score grid + the denominator.
            pv = [psp.tile([128, 512], F32, tag="pv", name=f"pv{dj}", bufs=4)
                  for dj in range(4)]

            # ---- PE warm-up: keep the tensor engine continuously busy while
            # the input DMAs stream so its p-state reaches full clock before
            # the first real matmul (results discarded; pv[0] is reset by
            # vmm's start=True later).
            for w in range(18):
                nc.tensor.matmul(pv[0][:, 0:256], warm[:, 0:128], warm[:],
                                 start=True, stop=True)

            # ---- span masks-matmul (k=0 half only) ----
            # ps[d, (dj, comp, slot)] = sum_s x[s, d] * mask[s, comp, slot]
            sc = nc.named_scope("spanmm"); sc.__enter__()
            evac = [nc.vector.tensor_copy,
                    lambda dst, src: nc.scalar.copy(dst, src)]
            for j0 in range(2):
                for l in range(SEN):
                    ps = psp.tile([128, 2, 3, G], F32, tag="psm", bufs=3)
                    for dj in range(2):
                        j = j0 * 2 + dj
                        nc.tensor.matmul(
                            ps[:, dj], x_sb[:, 0, l, j * 128:(j + 1) * 128],
                            mk_sb[:, 0:3, l * G:(l + 1) * G],
                            start=True, stop=True)
                    dstB = spanB[:, j0 * 6:(j0 + 1) * 6, l * G:(l + 1) * G]
                    evac[(l * 2 + j0) % 2](dstB, ps[:])
            sc.__exit__(None, None, None)

            # ---- v = span @ att_w (fp8 stationary, bf16 moving) ----
            sc = nc.named_scope("vmm"); sc.__enter__()
            for half in range(2):
                for dj in range(4):
                    for kk in range(6 * half, 6 * half + 6):
                        nc.tensor.matmul(
                            pv[dj][:, 0:C],
                            aw_sb[:, kk // 2, kk % 2, dj * 128:(dj + 1) * 128],
                            spanB[:, kk, :],
                            start=(kk == 0), stop=(kk == 11))
            for dj in range(4):
                evac[dj % 2](v_sb[:, dj, :], pv[dj][:, 0:C])
            sc.__exit__(None, None, None)

            # ---- fused c + span-tag-logits: out rows = [c | pls0..4] ----
            sc = nc.named_scope("cmm"); sc.__enter__()
            po = psp.tile([6, C], F32, tag="psm", name="po", bufs=3)
            for kk in range(12):
                nc.tensor.matmul(po[:], awc_sb[:, kk, :], spanB[:, kk, :],
                                 start=(kk == 0), stop=(kk == 11))
            nc.scalar.copy(out1_sb[:], po[:])
            nc.vector.tensor_copy(c_rb[:], po[0:1, :])
            nc.sync.dma_start(out1_d.ap(), out1_sb[:])
            sc.__exit__(None, None, None)

            # ---- scores + masked softmax chain, interleaved by k ----
            sc = nc.named_scope("gts"); sc.__enter__()
            gt = psp.tile([128, 2, C], F32, tag="pcd", name="gt", bufs=1)
            cbc = psp.tile([128, C], F32, tag="psm", name="cbc", bufs=3)
            for k in range(2):
                for l in range(SEN):
                    for dj in range(4):
                        nc.tensor.matmul(
                            gt[:, k, l * G:(l + 1) * G],
                            memT[:, l, k, dj, :],
                            v_sb[:, dj, l * G:(l + 1) * G],
                            start=(dj == 0), stop=(dj == 3))
                if k == 0:
                    nc.tensor.matmul(cbc[:], ones128[:], c_rb[:],
                                     start=True, stop=True)
                sg = wp.tile([128, C], F32, tag="sg", name=f"sg{k}")
                th = wp.tile([128, C], F32, tag="th", name=f"th{k}")
                e0 = wp.tile([128, C], F32, tag="e0", name=f"e0{k}")
                nc.vector.tensor_tensor(sg[:], gt[:, k], pw_sb[:, k], op=ALU.mult)
                nc.vector.tensor_tensor(sg[:], sg[:], cbc[:], op=ALU.add)
                nc.scalar.activation(th[:], sg[:], ACT.Tanh)
                nc.scalar.activation(e0[:], th[:], ACT.Exp)
                nc.vector.tensor_tensor(wTu[:, k], e0[:], pw_sb[:, k],
                                        op=ALU.mult)
                nc.vector.tensor_tensor(uT[:, k], e0[:], kp_sb[:, k],
                                        op=ALU.mult)
            sc.__exit__(None, None, None)

            # keep the PE p-state hot through the softmax-chain bubble
            # (results discarded; pv[2] is dead after its v evacuation)
            for w in range(12):
                nc.tensor.matmul(pv[2][:, 0:256], warm[:, 0:128], warm[:],
                                 start=True, stop=True)

            # ---- softmax denominator (ships to host; host divides) ----
            pd = psp.tile([1, C], F32, tag="pcd", name="pd", bufs=1)
            for k in range(2):
                nc.tensor.matmul(pd[:], ones1[:], uT[:, k], start=(k == 0),
                                 stop=(k == 1))
            nc.vector.tensor_copy(out2_sb[:], pd[:])
            nc.sync.dma_start(out2_d.ap(), out2_sb[:])

            # ---- mix[slot, d] = sum_{s,k} wTu[s, k, slot] x[s, k, d] ----
            # stationary = wTu block (48 slot cols), moving = full x row;
            # sentence l lands at partitions (l%2)*64 .. +48 of tile l//2.
            sc = nc.named_scope("mix"); sc.__enter__()
            pmix = [psp.tile([128, 512], F32, tag="pv", name=f"pmix{t}", bufs=4)
                    for t in range(2)]
            for t in range(2):
                for l in (2 * t, 2 * t + 1):
                    for k in range(2):
                        nc.tensor.matmul(
                            pmix[t][(l % 2) * 64:(l % 2) * 64 + G, :],
                            wTu[:, k, l * G:(l + 1) * G],
                            x_sb[:, k, l, :],
                            start=(k == 0), stop=(k == 1))
                for h in range(2):
                    evac[h](mix_sb[:, t, h * 256:(h + 1) * 256],
                            pmix[t][:, h * 256:(h + 1) * 256])
                nc.sync.dma_start(out3_d.ap()[:, t, :], mix_sb[:, t, :])
            sc.__exit__(None, None, None)

    nc.compile()
    return nc


def _row_of(kk, p):
    """att row for spanT chunk kk at partition p: comp*D + j*128 + p."""
    return (kk % 3) * D + (kk // 3) * 128 + p


def _host_prep(en_output, lengths, frag_b, frag_s, frag_e, att_w, att_b,
               tag_w, tag_b):
    """Shard + relayout inputs.  Returns (in_maps, assign, overflow)."""
    seq = float(lengths[0])
    p = np.arange(128)
    kk12 = np.arange(12)
    rows = _row_of(kk12[None, :], p[:, None])            # [128, 12]
    aw_np = np.ascontiguousarray(
        att_w[rows].reshape(128, 6, 2, D)).astype(NPF8)
    awc = np.empty((128, 12, 6), np.float32)
    awc[:, :, 0] = att_b[rows]
    awc[:, :, 1:] = tag_w[:, rows].transpose(1, 2, 0)    # [128, 12, T]
    awc_np = np.ascontiguousarray(awc).astype(NPBF)

    assign = np.full((F, 2), -1, dtype=np.int64)  # (core, slot) per fragment
    counts = np.zeros((NCORES, SEN), dtype=np.int64)
    overflow = []
    fs_slot = np.zeros((NCORES, C), np.float32)
    fm_slot = np.full((NCORES, C), -1.0, np.float32)
    ln_slot = np.full((NCORES, C), float(S), np.float32)

    for i in range(F):
        b = int(frag_b[i])
        s, e = int(frag_s[i]), int(frag_e[i])
        core, l = b // SEN, b % SEN
        k = counts[core, l]
        if k >= G or s >= KH or e > KH:
            overflow.append(i)
            continue
        counts[core, l] += 1
        slot = l * G + k
        assign[i] = (core, slot)
        fs_slot[core, slot] = s
        fm_slot[core, slot] = e - 1
        ln_slot[core, slot] = lengths[b]

    in_maps = []
    s0 = p.astype(np.float32)[:, None]                   # [128, 1]
    for core in range(NCORES):
        fs = fs_slot[core][None, :]
        fm = fm_slot[core][None, :]
        ln = ln_slot[core][None, :]
        mk = np.zeros((128, 3, C), np.float32)
        mk[:, 0, :] = s0 == fs
        mk[:, 1, :] = (s0 >= fs) & (s0 <= fm)
        mk[:, 2, :] = s0 == fm
        kp = np.empty((128, 2, C), np.float32)
        pw = np.empty((128, 2, C), np.float32)
        for k in range(2):
            s = k * KH + s0
            keep = (~((s >= fs) & (s <= fm))) & (s < ln)
            kp[:, k, :] = keep
            dis = np.where(s < fs, fs - s, np.where(s > fm, s - fm, seq))
            pw[:, k, :] = (1.0 - dis / seq) * keep
        xs = en_output[core * SEN:(core + 1) * SEN]      # [4, 256, 512]
        x_np = np.ascontiguousarray(
            xs.reshape(SEN, 2, 128, D).transpose(2, 1, 0, 3)).astype(NPBF)
        mem_np = np.ascontiguousarray(
            xs.reshape(SEN, 2, 128, 4, 128).transpose(4, 0, 1, 3, 2)).astype(NPF8)
        in_maps.append({
            "x": x_np, "memT": mem_np, "aw": aw_np, "awc": awc_np,
            "mk": np.ascontiguousarray(mk).astype(NPF8),
            "kp": np.ascontiguousarray(kp).astype(NPBF),
            "pw": np.ascontiguousarray(pw).astype(NPBF),
        })
    return in_maps, assign, overflow


def _host_fragment(en_output, lengths, s, e, b, att_w, att_b, tag_w, tag_b,
                   seq_len):
    """Numpy fallback for (vanishingly rare) slot-overflow fragments."""
    mem = en_output[b].astype(np.float64)
    ws = mem[s:e].sum(0)
    span = np.concatenate([mem[s], ws, mem[e - 1]])
    pos = np.arange(S)
    in_span = (pos >= s) & (pos < e)
    att_mask = in_span | (pos >= lengths[b])
    dis = np.where(pos < s, s - pos,
                   np.where(pos >= e, pos - e + 1, seq_len)).astype(np.float64)
    pwv = 1.0 - dis / seq_len
    fin = pwv[:, None] * mem
    v = span @ att_w.astype(np.float64)
    c = span @ att_b.astype(np.float64)
    sc = np.tanh(fin @ v + c)
    sc = np.where(att_mask, -1e4, sc)
    sc = sc - sc.max()
    a = np.exp(sc)
    a = a / a.sum()
    mix = a @ fin
    ms = np.concatenate([span, mix])
    lg = ms @ tag_w.astype(np.float64).T + tag_b.astype(np.float64)
    lg = lg - lg.max()
    return (lg - np.log(np.exp(lg).sum())).astype(np.float32)


def kernel(en_output, lengths, frag_b, frag_s, frag_e, att_w, att_b, tag_w,
           tag_b):
    global LAST_RESULT
    en_output = np.asarray(en_output, dtype=np.float32)
    lengths = np.asarray(lengths).astype(np.int64)
    frag_b = np.asarray(frag_b).astype(np.int64)
    frag_s = np.asarray(frag_s).astype(np.int64)
    frag_e = np.asarray(frag_e).astype(np.int64)
    att_w = np.asarray(att_w, dtype=np.float32)
    att_b = np.asarray(att_b, dtype=np.float32)
    tag_w = np.asarray(tag_w, dtype=np.float32)
    tag_b = np.asarray(tag_b, dtype=np.float32)

    if "nc" not in _compiled:
        _compiled["nc"] = _build()
    nc = _compiled["nc"]

    in_maps, assign, overflow = _host_prep(
        en_output, lengths, frag_b, frag_s, frag_e, att_w, att_b, tag_w, tag_b)

    res = run_bass_kernel_spmd(nc, in_maps, core_ids=list(range(NCORES)),
                               trace=TRACE)
    LAST_RESULT = res

    tw_mix = tag_w[:, 3 * D:]                            # [T, D]
    out = np.empty((F, T), dtype=np.float32)
    per_core = []
    for i in range(NCORES):
        o1 = np.asarray(res.results[i]["out1"], np.float32)     # [6, C]
        dn = np.asarray(res.results[i]["out2"], np.float32)[0]  # [C]
        mx = np.asarray(res.results[i]["out3"], np.float32)     # [128, 2, D]
        pls = o1[1:6, :].T                                      # [C, T]
        mix = np.empty((C, D), np.float32)
        for l in range(SEN):
            mix[l * G:(l + 1) * G] = mx[(l % 2) * 64:(l % 2) * 64 + G, l // 2]
        lg = pls + (mix @ tw_mix.T) / dn[:, None] + tag_b[None, :]
        m = lg.max(axis=1, keepdims=True)
        ls = lg - m
        per_core.append(ls - np.log(np.exp(ls).sum(axis=1, keepdims=True)))
    cores = assign[:, 0]
    slots = assign[:, 1]
    for core in range(NCORES):
        sel = cores == core
        out[sel] = per_core[core][slots[sel]]
    seq_len = float(lengths[0])
    for i in overflow:
        out[i] = _host_fragment(en_output, lengths, int(frag_s[i]),
                                int(frag_e[i]), int(frag_b[i]), att_w, att_b,
                                tag_w, tag_b, seq_len)
    return out
